# revision 12
# baseline (speedup 1.0000x reference)
"""LocalPatchAttention Trainium2 kernel (v2).

Data-parallel over batch B=8 across 8 NeuronCores (one image per core).

Host folds: q -> bf16; V-path (LayerNorm(v) @ vW.T + vb, scaled 1/4) computed
on host like the other parameter folds; attention matrix A = scale*(g.qW^T)K^T
and its bias; 3x3 conv weights pre-paired for DoubleRow fp8 matmuls (scaled
4x to keep e4m3 in normal range).

Per-core pipeline over 64 tiles of 4 image rows, each tile a [128, 512] bf16
SBUF tensor with partitions = (row-parity s, channel) and free = (row-pair j,
x):

  stats:  q^2 on GPSIMD; column sums of q and q^2 via two PE matmuls against
          a ones pattern -> [2, 1024] PSUM; one ACT copy to SBUF; eight tiny
          PE transposes pack per-pixel stats into a [128, 512] PSUM collector.
  batch:  every 16 tiles, one short DVE/ACT chain turns collected sums into
          rsqrt(var+eps) and mean*rsqrt columns (per-pixel, partition-major).
  attn:   eight PE transposes -> t1 [128px, 64ch]; DVE tensor_scalar applies
          LN using stat columns; four merged PE transposes back -> [128, 512]
          parity-packed xhT; one copy to SBUF; two logits matmuls against A;
          two ACT sigmoids (conv bias folded into the sigmoid bias); two
          GPSIMD multiplies with broadcast V -> fp8 rows in a contiguous
          258-row x_attn buffer (zero padding rows at both ends).
  conv:   PSUM preloaded with q via an identity matmul (residual for free),
          conv bias via one rank-1 matmul, then 12 fp8 DoubleRow matmuls
          (two 3x3 taps contracted per instruction); one ACT copy out; DMA.
"""

import numpy as np
import ml_dtypes

import concourse.bass as bass
import concourse.bacc as bacc
import concourse.tile as tile
from concourse import mybir
from concourse.bass_utils import run_bass_kernel_spmd

F32 = mybir.dt.float32
BF16 = mybir.dt.bfloat16
FP8 = mybir.dt.float8e4
AF = mybir.ActivationFunctionType
ALU = mybir.AluOpType
EPS = 1e-5
NPBF16 = ml_dtypes.bfloat16
NPFP8 = ml_dtypes.float8_e4m3

_CACHE = {}

import os
NT = int(os.environ.get("K_NT", "64"))
BATCH = int(os.environ.get("K_BATCH", "16"))
NB = NT // BATCH   # batches
CW_SCALE = 4.0     # fp8 conv weight upscale; V carries 1/CW_SCALE


def _build_nc(stage=99):
    nc = bacc.Bacc()
    q_d = nc.declare_dram_parameter("q", [128, 32768], BF16, isOutput=False)
    V_d = nc.declare_dram_parameter("Vf", [128, 4096], BF16, isOutput=False)
    A2a_d = nc.declare_dram_parameter("A2a", [128, 128], BF16, isOutput=False)
    A2b_d = nc.declare_dram_parameter("A2b", [128, 128], BF16, isOutput=False)
    cb_d = nc.declare_dram_parameter("cbias", [128, 1], F32, isOutput=False)
    cwt_d = nc.declare_dram_parameter("cwt2", [128, 1536], FP8, isOutput=False)
    cbb_d = nc.declare_dram_parameter("cbb", [1, 128], BF16, isOutput=False)
    i128_d = nc.declare_dram_parameter("i128", [128, 128], BF16, isOutput=False)
    i2_d = nc.declare_dram_parameter("i2", [2, 2], BF16, isOutput=False)
    on2_d = nc.declare_dram_parameter("ones2", [128, 2], BF16, isOutput=False)
    on5_d = nc.declare_dram_parameter("ones512", [1, 512], BF16, isOutput=False)
    out_d = nc.declare_dram_parameter("out", [128, 32768], F32, isOutput=True)

    with tile.TileContext(nc) as tc, \
         tc.tile_pool(name="const", bufs=1) as cpool, \
         tc.tile_pool(name="qb", bufs=24) as qb_pool, \
         tc.tile_pool(name="qsq", bufs=3) as qsq_pool, \
         tc.tile_pool(name="uwsb", bufs=3) as uw_pool, \
         tc.tile_pool(name="xh", bufs=6) as xh_pool, \
         tc.tile_pool(name="xhT", bufs=2) as xhT_pool, \
         tc.tile_pool(name="sig", bufs=4) as sig_pool, \
         tc.tile_pool(name="ot", bufs=3) as ot_pool, \
         tc.tile_pool(name="bch", bufs=2) as bch_pool, \
         tc.tile_pool(name="ps_uw", bufs=1, space="PSUM") as ps_uw, \
         tc.tile_pool(name="ps_coll", bufs=1, space="PSUM") as ps_coll, \
         tc.tile_pool(name="ps_t1", bufs=1, space="PSUM") as ps_t1, \
         tc.tile_pool(name="ps_xhT", bufs=1, space="PSUM") as ps_xhT, \
         tc.tile_pool(name="ps_lg", bufs=1, space="PSUM") as ps_lg, \
         tc.tile_pool(name="ps_cv", bufs=2, space="PSUM") as ps_cv:

        def const_tile(shape, dtype, tag, src):
            t = cpool.tile(shape, dtype, tag=tag)
            nc.sync.dma_start(out=t, in_=src[:, :])
            return t

        V_sb = const_tile([128, 4096], BF16, "V", V_d)
        A2a_sb = const_tile([128, 128], BF16, "A2a", A2a_d)
        A2b_sb = const_tile([128, 128], BF16, "A2b", A2b_d)
        cb_sb = const_tile([128, 1], F32, "cb", cb_d)
        cwt_sb = const_tile([128, 1536], FP8, "cwt", cwt_d)
        cbb_sb = const_tile([1, 128], BF16, "cbb", cbb_d)
        i128_sb = const_tile([128, 128], BF16, "i128", i128_d)
        i2_sb = const_tile([2, 2], BF16, "i2", i2_d)
        on2_sb = const_tile([128, 2], BF16, "on2", on2_d)
        on5_sb = const_tile([1, 512], BF16, "on5", on5_d)

        # persistent stat tables and the x_attn row buffer (258 slots)
        rr_sb = cpool.tile([128, 512], F32, tag="rr")
        murr_sb = cpool.tile([128, 512], F32, tag="murr")
        srow = cpool.tile([128, 258 * 256], FP8, tag="srow")
        srow3 = srow.rearrange("p (r x) -> p r x", x=256)
        nc.vector.memset(srow3[:, 0, :], 0.0)
        nc.vector.memset(srow3[:, 257, :], 0.0)

        # collector [128, 512]: two 256-col halves alternate between batches
        coll = ps_coll.tile([128, 512], F32, tag="coll")

        qbs = {}

        def stats(t):
            k = t % BATCH
            b = t // BATCH
            qb = qb_pool.tile([128, 512], BF16, tag="qb")
            nc.sync.dma_start(out=qb, in_=q_d[:, 512 * t:512 * (t + 1)])
            qbs[t] = qb
            qsq = qsq_pool.tile([128, 512], BF16, tag="qsq")
            nc.gpsimd.tensor_tensor(qsq, qb, qb, ALU.mult)
            uw = ps_uw.tile([2, 1024], F32, tag="uw")
            nc.tensor.matmul(uw[:, 0:512], on2_sb, qb, start=True, stop=True)
            nc.tensor.matmul(uw[:, 512:1024], on2_sb, qsq, start=True, stop=True)
            uwsb = uw_pool.tile([2, 1024], BF16, tag="uwsb")
            nc.scalar.copy(uwsb, uw)
            base = 256 * (b % 2) + 16 * k
            for jc in range(4):
                nc.tensor.matmul(coll[:, base + 2 * jc: base + 2 * jc + 2],
                                 uwsb[:, 128 * jc:128 * (jc + 1)], i2_sb,
                                 start=True, stop=True)
                nc.tensor.matmul(coll[:, base + 8 + 2 * jc: base + 10 + 2 * jc],
                                 uwsb[:, 512 + 128 * jc:512 + 128 * (jc + 1)],
                                 i2_sb, start=True, stop=True)

        def batch_chain(b):
            half = coll[:, 256 * (b % 2):256 * (b % 2) + 256]
            cv3 = half.rearrange("p (k d) -> p k d", d=16)
            u = cv3[:, :, 0:8]
            w = cv3[:, :, 8:16]
            sh = [128, 16, 8]
            mu = bch_pool.tile(sh, F32, tag="mu")
            nc.vector.tensor_scalar_mul(mu, u, 1.0 / 64)
            ew = bch_pool.tile(sh, F32, tag="ew")
            nc.vector.tensor_scalar_mul(ew, w, 1.0 / 64)
            m2 = bch_pool.tile(sh, F32, tag="m2")
            nc.vector.tensor_tensor(m2, mu, mu, ALU.mult)
            var = bch_pool.tile(sh, F32, tag="var")
            nc.vector.tensor_tensor(var, ew, m2, ALU.subtract)
            nc.vector.tensor_scalar_add(var, var, EPS)
            rec = bch_pool.tile(sh, F32, tag="rec")
            nc.vector.reciprocal(rec, var)
            rrs = rr_sb[:, 128 * b:128 * (b + 1)].rearrange(
                "p (k d) -> p k d", d=8)
            nc.scalar.activation(rrs, rec, AF.Sqrt)
            murrs = murr_sb[:, 128 * b:128 * (b + 1)].rearrange(
                "p (k d) -> p k d", d=8)
            nc.vector.tensor_tensor(murrs, mu, rrs, ALU.mult)

        def attn(t, sub=3):
            qb = qbs[t]
            t1 = ps_t1.tile([128, 512], F32, tag="t1")
            for j in range(2):
                for c in range(2):
                    for s in range(2):
                        idx = (j * 2 + c) * 2 + s
                        nc.tensor.matmul(
                            t1[:, 64 * idx:64 * (idx + 1)],
                            qb[:, j * 256 + c * 128: j * 256 + (c + 1) * 128],
                            i128_sb[:, 64 * s:64 * (s + 1)],
                            start=True, stop=True)
            if sub <= -2 or sub >= 30:
                ot = ot_pool.tile([128, 512], F32, tag="ot")
                nc.vector.tensor_copy(ot, t1)
                nc.sync.dma_start(out=out_d[:, 512 * t:512 * (t + 1)], in_=ot)
                return
            xhT = ps_xhT.tile([128, 512], F32, tag="xhT")
            for j in range(2):
                for c in range(2):
                    jc = j * 2 + c
                    xh2 = xh_pool.tile([128, 128], BF16, tag="xh2")
                    for s in range(2):
                        idx = jc * 2 + s
                        rcol = 8 * t + 2 * jc + s
                        nc.vector.tensor_scalar(
                            xh2[:, 64 * s:64 * (s + 1)],
                            t1[:, 64 * idx:64 * (idx + 1)],
                            rr_sb[:, rcol:rcol + 1],
                            murr_sb[:, rcol:rcol + 1],
                            ALU.mult, ALU.subtract)
                    if sub <= -1 and jc == 0:
                        ot = ot_pool.tile([128, 512], F32, tag="ot")
                        nc.scalar.copy(ot[:, 0:128], xh2)
                        nc.sync.dma_start(out=out_d[:, 512 * t:512 * t + 128],
                                          in_=ot[:, 0:128])
                    if sub > -1:
                        nc.tensor.matmul(xhT[:, 128 * jc:128 * (jc + 1)],
                                         xh2, i128_sb, start=True, stop=True)
            if sub <= -1:
                return
            xhTs = xhT_pool.tile([128, 512], BF16, tag="xhTs")
            nc.vector.tensor_copy(xhTs, xhT)
            if sub < 1:
                return
            for s in range(2):
                lg = ps_lg.tile([128, 512], F32, tag="lg")
                nc.tensor.matmul(lg, (A2a_sb, A2b_sb)[s], xhTs,
                                 start=True, stop=True)
                sig = sig_pool.tile([128, 512], BF16, tag="sig")
                nc.scalar.activation(sig, lg, AF.Sigmoid, bias=cb_sb[:, 0:1])
                if sub < 2:
                    continue
                # rows 4t+s and 4t+2+s -> slots 4t+s+1 (+2)
                slot = 4 * t + s + 1
                outap = srow3[:, slot:slot + 3:2, :].rearrange(
                    "p j (w f) -> p j w f", f=4)
                vb = V_sb[:, 64 * t:64 * (t + 1)].rearrange(
                    "p (o w) -> p o w ()", o=1).broadcast_to([128, 2, 64, 4])
                nc.gpsimd.tensor_tensor(
                    outap,
                    sig.rearrange("p (j w f) -> p j w f", j=2, f=4),
                    vb, ALU.mult)

        def conv(t):
            cv = ps_cv.tile([128, 512], F32, tag="cv")
            nc.tensor.matmul(cv, i128_sb, qbs.pop(t), start=True, stop=False)
            nc.tensor.matmul(cv, cbb_sb, on5_sb, start=False, stop=False)
            for d in range(3):
                for h in range(2):
                    wt = cwt_sb[:, (d * 2 + h) * 256:(d * 2 + h) * 256 + 256]
                    wt3 = wt.rearrange("p (k m) -> p k m", k=2)
                    for p in range(2):
                        slot = 4 * t + 2 * p + 2 * h
                        last = (d == 2 and h == 1 and p == 1)
                        rt = srow3[:, slot:slot + 2, :]
                        DR = mybir.MatmulPerfMode.DoubleRow
                        if d == 0:    # dx=1 center
                            nc.tensor.matmul(cv[:, 256 * p:256 * p + 256],
                                             wt3, rt, start=False, stop=last,
                                             perf_mode=DR)
                        elif d == 1:  # dx=0: out x gets in x-1
                            nc.tensor.matmul(cv[:, 256 * p + 1:256 * p + 256],
                                             wt3, rt[:, :, 0:255],
                                             start=False, stop=last,
                                             perf_mode=DR)
                        else:         # dx=2: out x gets in x+1
                            nc.tensor.matmul(cv[:, 256 * p:256 * p + 255],
                                             wt3, rt[:, :, 1:256],
                                             start=False, stop=last,
                                             perf_mode=DR)
            ot = ot_pool.tile([128, 512], F32, tag="ot")
            nc.scalar.copy(ot, cv)
            nc.sync.dma_start(out=out_d[:, 512 * t:512 * (t + 1)], in_=ot)

        if stage == 0:
            for t in range(NT):
                stats(t)
                if t % BATCH == BATCH - 1:
                    batch_chain(t // BATCH)
                ot = ot_pool.tile([128, 512], F32, tag="ot")
                nc.vector.tensor_copy(ot, qbs.pop(t))
                nc.sync.dma_start(out=out_d[:, 512 * t:512 * (t + 1)], in_=ot)
        elif stage in (1, 10, 11, 20, 21, 30, 31):
            sub = {20: -2, 21: -1, 10: 0, 11: 1, 1: 3, 30: 30, 31: 31}[stage]
            for t in range(NT):
                stats(t)
                if t % BATCH == BATCH - 1:
                    batch_chain(t // BATCH)
                if t >= BATCH:
                    attn(t - BATCH, sub)
                    ot = ot_pool.tile([128, 512], F32, tag="ot")
                    nc.vector.tensor_copy(ot, qbs.pop(t - BATCH))
                    nc.sync.dma_start(
                        out=out_d[:, 512 * (t - BATCH):512 * (t - BATCH + 1)], in_=ot)
            for t in range(NT - BATCH, NT):
                attn(t, sub)
                ot = ot_pool.tile([128, 512], F32, tag="ot")
                nc.vector.tensor_copy(ot, qbs.pop(t))
                nc.sync.dma_start(out=out_d[:, 512 * t:512 * (t + 1)], in_=ot)
        else:
            for t in range(NT):
                stats(t)
                if t % BATCH == BATCH - 1:
                    batch_chain(t // BATCH)
                if t >= BATCH:
                    attn(t - BATCH)
                if t >= BATCH + 1:
                    conv(t - BATCH - 1)
            for t in range(NT - BATCH, NT):
                attn(t)
                conv(t - 1)
            conv(NT - 1)

    nc.finalize()
    return nc


def _fold_weights(qW, qb, vW, vb, K, qn_g, qn_b, vn_g, vn_b, cW, cb):
    f = np.float32
    qW, qb, vW, vb, K = f(qW), f(qb), f(vW), f(vb), f(K)
    qn_g, qn_b, vn_g, vn_b, cW, cb = f(qn_g), f(qn_b), f(vn_g), f(vn_b), f(cW), f(cb)
    scale = np.float32(64.0 ** -0.5)
    qWf = qn_g[:, None] * qW.T                      # [c, co]
    bprime = qb + qW @ qn_b                         # [64]
    A = scale * (qWf @ K.T)                         # [64, 128]
    c_b = scale * (K @ bprime)                      # [128]
    # conv weights, DoubleRow-paired: block (d, h, k, m=(s,och))
    # tap t = -1 + 2h + k, dx order (1, 0, 2); ky = t + 1 - s
    cwt2 = np.zeros((128, 3, 2, 2, 128), np.float32)
    for d, dx in enumerate((1, 0, 2)):
        for h in range(2):
            for k in range(2):
                tt = -1 + 2 * h + k
                for s in range(2):
                    ky = tt + 1 - s
                    if 0 <= ky <= 2:
                        cwt2[:, d, h, k, 64 * s:64 * (s + 1)] = cW[:, :, ky, dx].T
    cwt2 *= CW_SCALE
    ones2 = np.stack([np.r_[np.ones(64), np.zeros(64)],
                      np.r_[np.zeros(64), np.ones(64)]], 1)
    return {
        "A2a": np.ascontiguousarray(
            np.concatenate([A, np.zeros((64, 128), np.float32)], 0).astype(NPBF16)),
        "A2b": np.ascontiguousarray(
            np.concatenate([np.zeros((64, 128), np.float32), A], 0).astype(NPBF16)),
        "cbias": np.ascontiguousarray(c_b.reshape(128, 1)),
        "cwt2": np.ascontiguousarray(cwt2.reshape(128, 1536).astype(NPFP8)),
        "cbb": np.ascontiguousarray(
            np.concatenate([cb, cb]).reshape(1, 128).astype(NPBF16)),
        "i128": np.eye(128, dtype=np.float32).astype(NPBF16),
        "i2": np.eye(2, dtype=np.float32).astype(NPBF16),
        "ones2": np.ascontiguousarray(ones2.astype(NPBF16)),
        "ones512": np.ones((1, 512), np.float32).astype(NPBF16),
        "vW": vW, "vb": vb, "vn_g": vn_g, "vn_b": vn_b,
    }


def _fold_v(v_i, vW, vb, vn_g, vn_b):
    # v_i [128, 64, 64] -> V [128, 4096] bf16, scaled by 1/CW_SCALE
    x = np.float32(v_i).reshape(128, 4096)
    mu = x.mean(0, keepdims=True)
    var = x.var(0, keepdims=True)
    vh = (x - mu) / np.sqrt(var + EPS) * vn_g[:, None] + vn_b[:, None]
    V = vW @ vh + vb[:, None]
    return np.ascontiguousarray((V / CW_SCALE).astype(NPBF16))


def _make_inmaps(q, v, qW, qb, vW, vb, K, qn_g, qn_b, vn_g, vn_b, cW, cb):
    base = _fold_weights(qW, qb, vW, vb, K, qn_g, qn_b, vn_g, vn_b, cW, cb)
    vWf, vbf = base.pop("vW"), base.pop("vb")
    vng, vnb = base.pop("vn_g"), base.pop("vn_b")
    in_maps = []
    for i in range(8):
        m = dict(base)
        qi = np.float32(q[i]).reshape(64, 64, 2, 2, 256)  # c, t, j, s, x
        qi = qi.transpose(3, 0, 1, 2, 4)                      # s, c, t, j, x
        m["q"] = np.ascontiguousarray(qi.reshape(128, 32768).astype(NPBF16))
        m["Vf"] = _fold_v(v[i], vWf, vbf, vng, vnb)
        in_maps.append(m)
    return in_maps


def _run(in_maps, trace=False, **kw):
    if "nc" not in _CACHE:
        _CACHE["nc"] = _build_nc()
    return run_bass_kernel_spmd(_CACHE["nc"], in_maps, list(range(8)),
                                trace=trace, **kw)


def kernel(q, v, qW, qb, vW, vb, K, qn_g, qn_b, vn_g, vn_b, cW, cb):
    in_maps = _make_inmaps(q, v, qW, qb, vW, vb, K,
                           qn_g, qn_b, vn_g, vn_b, cW, cb)
    res = _run(in_maps)
    outs = []
    for r in res.results:
        o = np.asarray(r["out"], np.float32).reshape(2, 64, 64, 2, 256)
        # (s, c, t, p, x) -> (c, t, p, s, x) = (c, 256 rows, 256 cols)
        o = o.transpose(1, 2, 3, 0, 4).reshape(64, 256, 256)
        outs.append(o)
    return np.stack(outs)


# revision 14
# speedup vs baseline: 1.1182x; 1.1182x over previous
"""LocalPatchAttention Trainium2 kernel (v3).

Data-parallel over batch B=8 across 8 NeuronCores (one image per core).

Host folds: q packed to the device layout in bf16; the tiny V-path
(LayerNorm(v) @ vW.T + vb, scaled 1/4) folded on host like the other
parameters; attention matrix A = scale*(g.qW^T)K^T (zero-padded per row
parity) and its bias; 3x3 conv weights pre-paired for fp8 DoubleRow matmuls
(scaled 4x to stay in e4m3 normal range), with the conv bias folded in as an
extra DoubleRow k-tile against a constant 0.25 row.

Per-core pipeline over 64 tiles of 4 image rows, each tile a [128, 512] bf16
SBUF tensor with partitions = (row-parity s, channel) and free = (row-pair j,
x):

  stats:  q^2 on GPSIMD; column sums of q and q^2 via two PE matmuls against
          a ones pattern -> [2, 1024] PSUM; one ACT copy to SBUF; eight tiny
          PE transposes pack per-pixel stats into a [128, 512] PSUM collector.
  batch:  every 8 tiles a short DVE chain turns the collected sums into
          rsqrt(var+eps) (Newton iteration, no ACT tables) and mean*rsqrt.
  attn:   eight PE transposes -> t1 [128px, 64ch] PSUM; one ACT copy to SBUF;
          LN applied per chunk by GPSIMD/DVE tensor_scalar with per-pixel stat
          columns; four merged PE transposes back -> parity-packed xhT PSUM;
          one DVE copy to SBUF; two logits matmuls; two ACT sigmoids (bias
          folded); two GPSIMD multiplies with broadcast V -> fp8 rows of a
          contiguous 259-slot x_attn buffer (ones/zero pad slots).
  conv:   PSUM preloaded with q via an identity matmul (the residual add for
          free), then 14 fp8 DoubleRow matmuls (two 3x3 taps or bias+tap per
          instruction); ACT/DVE copy out (alternating); DMA.
"""

import numpy as np
import ml_dtypes

import concourse.bass as bass
import concourse.bacc as bacc
import concourse.tile as tile
from concourse import mybir
from concourse.bass_utils import run_bass_kernel_spmd

F32 = mybir.dt.float32
I32 = mybir.dt.int32
BF16 = mybir.dt.bfloat16
FP8 = mybir.dt.float8e4
AF = mybir.ActivationFunctionType
ALU = mybir.AluOpType
EPS = 1e-5
NPBF16 = ml_dtypes.bfloat16
NPFP8 = ml_dtypes.float8_e4m3

_CACHE = {}

NT = 64           # tiles per core (4 image rows each)
BATCH = 8         # tiles per stats batch
CW_SCALE = 4.0    # fp8 conv weight upscale; V and the bias row carry 1/4
XH_POOL = 6       # xh chunks on GPSIMD (rest on DVE)

# DoubleRow pair list: (dx-index, tapA, tapB); "B" = bias row (slot 0),
# "Z" = zero row (slot 1); tap t -> slot base+2+t with base = 4*tile + 2*p.
DR_PAIRS = [(0, "B", -1), (0, 0, 1), (0, "Z", 2),
            (1, -1, 0), (1, 1, 2),
            (2, -1, 0), (2, 1, 2)]


def _build_nc():
    nc = bacc.Bacc()
    q_d = nc.declare_dram_parameter("q", [128, 32768], BF16, isOutput=False)
    V_d = nc.declare_dram_parameter("Vf", [128, 4096], BF16, isOutput=False)
    A2a_d = nc.declare_dram_parameter("A2a", [128, 128], BF16, isOutput=False)
    A2b_d = nc.declare_dram_parameter("A2b", [128, 128], BF16, isOutput=False)
    cb_d = nc.declare_dram_parameter("cbias", [128, 1], F32, isOutput=False)
    cwt_d = nc.declare_dram_parameter("cwt3", [128, 1792], FP8, isOutput=False)
    i128_d = nc.declare_dram_parameter("i128", [128, 128], BF16, isOutput=False)
    i2_d = nc.declare_dram_parameter("i2", [2, 2], BF16, isOutput=False)
    on2_d = nc.declare_dram_parameter("ones2", [128, 2], BF16, isOutput=False)
    out_d = nc.declare_dram_parameter("out", [128, 32768], F32, isOutput=True)

    with tile.TileContext(nc) as tc, \
         tc.tile_pool(name="const", bufs=1) as cpool, \
         tc.tile_pool(name="qb", bufs=16) as qb_pool, \
         tc.tile_pool(name="qsq", bufs=3) as qsq_pool, \
         tc.tile_pool(name="uwsb", bufs=3) as uw_pool, \
         tc.tile_pool(name="t1s", bufs=3) as t1s_pool, \
         tc.tile_pool(name="xh", bufs=6) as xh_pool, \
         tc.tile_pool(name="xhT", bufs=2) as xhT_pool, \
         tc.tile_pool(name="sig", bufs=4) as sig_pool, \
         tc.tile_pool(name="ot", bufs=3) as ot_pool, \
         tc.tile_pool(name="bch", bufs=2) as bch_pool, \
         tc.tile_pool(name="ps_uw", bufs=1, space="PSUM") as ps_uw, \
         tc.tile_pool(name="ps_coll", bufs=1, space="PSUM") as ps_coll, \
         tc.tile_pool(name="ps_t1", bufs=1, space="PSUM") as ps_t1, \
         tc.tile_pool(name="ps_xhT", bufs=1, space="PSUM") as ps_xhT, \
         tc.tile_pool(name="ps_lg", bufs=1, space="PSUM") as ps_lg, \
         tc.tile_pool(name="ps_cv", bufs=2, space="PSUM") as ps_cv:

        def const_tile(shape, dtype, tag, src):
            t = cpool.tile(shape, dtype, tag=tag)
            nc.sync.dma_start(out=t, in_=src[:, :])
            return t

        V_sb = const_tile([128, 4096], BF16, "V", V_d)
        A2a_sb = const_tile([128, 128], BF16, "A2a", A2a_d)
        A2b_sb = const_tile([128, 128], BF16, "A2b", A2b_d)
        cb_sb = const_tile([128, 1], F32, "cb", cb_d)
        cwt_sb = const_tile([128, 1792], FP8, "cwt", cwt_d)
        i128_sb = const_tile([128, 128], BF16, "i128", i128_d)
        i2_sb = const_tile([2, 2], BF16, "i2", i2_d)
        on2_sb = const_tile([128, 2], BF16, "on2", on2_d)

        # int constants for the Newton rsqrt seed
        magic_sb = cpool.tile([128, 64], I32, tag="magic")
        nc.vector.memset(magic_sb, 0x5F3759DF)
        one_sb = cpool.tile([128, 1], I32, tag="one1")
        nc.vector.memset(one_sb, 1)

        # persistent stat tables and the x_attn row buffer
        # slots: 0 = 0.25 (bias row), 1 = zero, row r -> slot r+2, 258 = zero
        rr_sb = cpool.tile([128, 512], F32, tag="rr")
        murr_sb = cpool.tile([128, 512], F32, tag="murr")
        srow = cpool.tile([128, 259 * 256], FP8, tag="srow")
        srow3 = srow.rearrange("p (r x) -> p r x", x=256)
        nc.vector.memset(srow3[:, 0, :], 0.25)
        nc.vector.memset(srow3[:, 1, :], 0.0)
        nc.vector.memset(srow3[:, 258, :], 0.0)

        # collector [128, 512]: 128-col quarters rotate across batches
        coll = ps_coll.tile([128, 512], F32, tag="coll")

        qbs = {}

        def stats(t):
            k = t % BATCH
            b = t // BATCH
            qb = qb_pool.tile([128, 512], BF16, tag="qb")
            nc.sync.dma_start(out=qb, in_=q_d[:, 512 * t:512 * (t + 1)])
            qbs[t] = qb
            qsq = qsq_pool.tile([128, 512], BF16, tag="qsq")
            nc.gpsimd.tensor_tensor(qsq, qb, qb, ALU.mult)
            uw = ps_uw.tile([2, 1024], F32, tag="uw")
            nc.tensor.matmul(uw[:, 0:512], on2_sb, qb, start=True, stop=True)
            nc.tensor.matmul(uw[:, 512:1024], on2_sb, qsq, start=True, stop=True)
            uwsb = uw_pool.tile([2, 1024], BF16, tag="uwsb")
            nc.scalar.copy(uwsb, uw)
            base = 128 * (b % 4) + 16 * k
            for jc in range(4):
                nc.tensor.matmul(coll[:, base + 2 * jc: base + 2 * jc + 2],
                                 uwsb[:, 128 * jc:128 * (jc + 1)], i2_sb,
                                 start=True, stop=True)
                nc.tensor.matmul(coll[:, base + 8 + 2 * jc: base + 10 + 2 * jc],
                                 uwsb[:, 512 + 128 * jc:512 + 128 * (jc + 1)],
                                 i2_sb, start=True, stop=True)

        def batch_chain(b):
            quarter = coll[:, 128 * (b % 4):128 * (b % 4) + 128]
            cv3 = quarter.rearrange("p (k d) -> p k d", d=16)
            u = cv3[:, :, 0:8]
            w = cv3[:, :, 8:16]
            sh = [128, BATCH, 8]
            mu = bch_pool.tile(sh, F32, tag="mu")
            nc.vector.tensor_scalar_mul(mu, u, 1.0 / 64)
            ew = bch_pool.tile(sh, F32, tag="ew")
            nc.vector.tensor_scalar_mul(ew, w, 1.0 / 64)
            m2 = bch_pool.tile(sh, F32, tag="m2")
            nc.vector.tensor_tensor(m2, mu, mu, ALU.mult)
            var = bch_pool.tile(sh, F32, tag="var")
            nc.vector.tensor_tensor(var, ew, m2, ALU.subtract)
            nc.vector.tensor_scalar_add(var, var, EPS)
            # Newton rsqrt: y0 via the int bit trick, then two iterations
            y = bch_pool.tile(sh, F32, tag="y")
            yi = y.bitcast(I32)
            nc.vector.tensor_scalar(yi, var.bitcast(I32), one_sb[:, 0:1], None,
                                    ALU.logical_shift_right)
            nc.vector.tensor_tensor(
                yi, magic_sb.rearrange("p (k d) -> p k d", d=8)[:, 0:BATCH],
                yi, ALU.subtract)
            rrs = rr_sb[:, 64 * b:64 * (b + 1)].rearrange("p (k d) -> p k d", d=8)
            h = bch_pool.tile(sh, F32, tag="h")
            for it in range(2):
                nc.vector.tensor_tensor(h, y, y, ALU.mult)
                nc.vector.tensor_tensor(h, var, h, ALU.mult)
                nc.vector.tensor_scalar(h, h, -0.5, 1.5, ALU.mult, ALU.add)
                nc.vector.tensor_tensor(rrs if it == 1 else y, y, h, ALU.mult)
            murrs = murr_sb[:, 64 * b:64 * (b + 1)].rearrange(
                "p (k d) -> p k d", d=8)
            nc.vector.tensor_tensor(murrs, mu, rrs, ALU.mult)

        def attn(t):
            qb = qbs[t]
            t1 = ps_t1.tile([128, 512], F32, tag="t1")
            for j in range(2):
                for c in range(2):
                    for s in range(2):
                        idx = (j * 2 + c) * 2 + s
                        nc.tensor.matmul(
                            t1[:, 64 * idx:64 * (idx + 1)],
                            qb[:, j * 256 + c * 128: j * 256 + (c + 1) * 128],
                            i128_sb[:, 64 * s:64 * (s + 1)],
                            start=True, stop=True)
            t1s = t1s_pool.tile([128, 512], BF16, tag="t1s")
            nc.scalar.copy(t1s, t1)
            xhT = ps_xhT.tile([128, 512], F32, tag="xhT")
            for j in range(2):
                for c in range(2):
                    jc = j * 2 + c
                    xh2 = xh_pool.tile([128, 128], BF16, tag="xh2")
                    for s in range(2):
                        idx = jc * 2 + s
                        rcol = 8 * t + 2 * jc + s
                        eng = nc.gpsimd if idx < XH_POOL else nc.vector
                        eng.tensor_scalar(
                            xh2[:, 64 * s:64 * (s + 1)],
                            t1s[:, 64 * idx:64 * (idx + 1)],
                            rr_sb[:, rcol:rcol + 1],
                            murr_sb[:, rcol:rcol + 1],
                            ALU.mult, ALU.subtract)
                    nc.tensor.matmul(xhT[:, 128 * jc:128 * (jc + 1)],
                                     xh2, i128_sb, start=True, stop=True)
            xhTs = xhT_pool.tile([128, 512], BF16, tag="xhTs")
            nc.vector.tensor_copy(xhTs, xhT)
            for s in range(2):
                lg = ps_lg.tile([128, 512], F32, tag="lg")
                nc.tensor.matmul(lg, (A2a_sb, A2b_sb)[s], xhTs,
                                 start=True, stop=True)
                sig = sig_pool.tile([128, 512], BF16, tag="sig")
                nc.scalar.activation(sig, lg, AF.Sigmoid, bias=cb_sb[:, 0:1])
                # rows 4t+s and 4t+2+s -> slots 4t+s+2 (+2)
                slot = 4 * t + s + 2
                outap = srow3[:, slot:slot + 3:2, :].rearrange(
                    "p j (w f) -> p j w f", f=4)
                vb = V_sb[:, 64 * t:64 * (t + 1)].rearrange(
                    "p (o w) -> p o w ()", o=1).broadcast_to([128, 2, 64, 4])
                nc.gpsimd.tensor_tensor(
                    outap,
                    sig.rearrange("p (j w f) -> p j w f", j=2, f=4),
                    vb, ALU.mult)

        def conv(t):
            cv = ps_cv.tile([128, 512], F32, tag="cv")
            nc.tensor.matmul(cv, i128_sb, qbs.pop(t), start=True, stop=False)
            DR = mybir.MatmulPerfMode.DoubleRow
            for pi, (d, ta, tb) in enumerate(DR_PAIRS):
                wt3 = cwt_sb[:, pi * 256:(pi + 1) * 256].rearrange(
                    "p (k m) -> p k m", k=2)
                for p in range(2):
                    base = 4 * t + 2 * p
                    sa = 0 if ta == "B" else (1 if ta == "Z" else base + 2 + ta)
                    sb_ = base + 2 + tb
                    last = (pi == len(DR_PAIRS) - 1 and p == 1)
                    step = sb_ - sa
                    rt = srow3[:, sa:sb_ + 1:step, :]
                    if d == 0:    # dx=1 center
                        nc.tensor.matmul(cv[:, 256 * p:256 * p + 256],
                                         wt3, rt, start=False, stop=last,
                                         perf_mode=DR)
                    elif d == 1:  # dx=0: out x gets in x-1
                        nc.tensor.matmul(cv[:, 256 * p + 1:256 * p + 256],
                                         wt3, rt[:, :, 0:255],
                                         start=False, stop=last, perf_mode=DR)
                    else:         # dx=2: out x gets in x+1
                        nc.tensor.matmul(cv[:, 256 * p:256 * p + 255],
                                         wt3, rt[:, :, 1:256],
                                         start=False, stop=last, perf_mode=DR)
            ot = ot_pool.tile([128, 512], F32, tag="ot")
            if t % 2 == 0:
                nc.scalar.copy(ot, cv)
            else:
                nc.vector.tensor_copy(ot, cv)
            nc.sync.dma_start(out=out_d[:, 512 * t:512 * (t + 1)], in_=ot)

        for t in range(NT):
            stats(t)
            if t % BATCH == BATCH - 1:
                batch_chain(t // BATCH)
            if t >= BATCH:
                attn(t - BATCH)
            if t >= BATCH + 1:
                conv(t - BATCH - 1)
        for t in range(NT - BATCH, NT):
            attn(t)
            conv(t - 1)
        conv(NT - 1)

    nc.finalize()
    return nc


def _fold_weights(qW, qb, vW, vb, K, qn_g, qn_b, vn_g, vn_b, cW, cb):
    f = np.float32
    qW, qb, vW, vb, K = f(qW), f(qb), f(vW), f(vb), f(K)
    qn_g, qn_b, vn_g, vn_b, cW, cb = f(qn_g), f(qn_b), f(vn_g), f(vn_b), f(cW), f(cb)
    scale = np.float32(64.0 ** -0.5)
    qWf = qn_g[:, None] * qW.T                      # [c, co]
    bprime = qb + qW @ qn_b                         # [64]
    A = scale * (qWf @ K.T)                         # [64, 128]
    c_b = scale * (K @ bprime)                      # [128]

    cb2 = np.concatenate([cb, cb])                  # [128] conv bias (s, och)
    dxs = (1, 0, 2)
    cwt3 = np.zeros((128, 7, 2, 128), np.float32)
    for pi, (d, ta, tb) in enumerate(DR_PAIRS):
        for ki, tap in enumerate((ta, tb)):
            if tap == "Z":
                continue
            if tap == "B":
                # bias row holds 0.25: w[0, m] = cb2[m] * CW_SCALE
                cwt3[0, pi, ki, :] = cb2 * CW_SCALE
                continue
            for s in range(2):
                ky = tap + 1 - s
                if 0 <= ky <= 2:
                    cwt3[:, pi, ki, 64 * s:64 * (s + 1)] = \
                        cW[:, :, ky, dxs[d]].T * CW_SCALE
    return {
        "A2a": np.ascontiguousarray(
            np.concatenate([A, np.zeros((64, 128), np.float32)], 0).astype(NPBF16)),
        "A2b": np.ascontiguousarray(
            np.concatenate([np.zeros((64, 128), np.float32), A], 0).astype(NPBF16)),
        "cbias": np.ascontiguousarray(c_b.reshape(128, 1)),
        "cwt3": np.ascontiguousarray(cwt3.reshape(128, 1792).astype(NPFP8)),
        "i128": np.eye(128, dtype=np.float32).astype(NPBF16),
        "i2": np.eye(2, dtype=np.float32).astype(NPBF16),
        "ones2": np.ascontiguousarray(
            np.stack([np.r_[np.ones(64), np.zeros(64)],
                      np.r_[np.zeros(64), np.ones(64)]], 1).astype(NPBF16)),
        "vW": vW, "vb": vb, "vn_g": vn_g, "vn_b": vn_b,
    }


def _fold_v(v_i, vW, vb, vn_g, vn_b):
    x = np.float32(v_i).reshape(128, 4096)
    mu = x.mean(0, keepdims=True)
    var = x.var(0, keepdims=True)
    vh = (x - mu) / np.sqrt(var + EPS) * vn_g[:, None] + vn_b[:, None]
    V = vW @ vh + vb[:, None]
    return np.ascontiguousarray((V / CW_SCALE).astype(NPBF16))


def _make_inmaps(q, v, qW, qb, vW, vb, K, qn_g, qn_b, vn_g, vn_b, cW, cb):
    base = _fold_weights(qW, qb, vW, vb, K, qn_g, qn_b, vn_g, vn_b, cW, cb)
    vWf, vbf = base.pop("vW"), base.pop("vb")
    vng, vnb = base.pop("vn_g"), base.pop("vn_b")
    in_maps = []
    for i in range(8):
        m = dict(base)
        qi = np.float32(q[i]).reshape(64, 64, 2, 2, 256)  # c, t, j, s, x
        qi = qi.transpose(3, 0, 1, 2, 4)                  # s, c, t, j, x
        m["q"] = np.ascontiguousarray(qi.reshape(128, 32768).astype(NPBF16))
        m["Vf"] = _fold_v(v[i], vWf, vbf, vng, vnb)
        in_maps.append(m)
    return in_maps


def _run(in_maps, trace=False, **kw):
    if "nc" not in _CACHE:
        _CACHE["nc"] = _build_nc()
    return run_bass_kernel_spmd(_CACHE["nc"], in_maps, list(range(8)),
                                trace=trace, **kw)


def kernel(q, v, qW, qb, vW, vb, K, qn_g, qn_b, vn_g, vn_b, cW, cb):
    in_maps = _make_inmaps(q, v, qW, qb, vW, vb, K,
                           qn_g, qn_b, vn_g, vn_b, cW, cb)
    res = _run(in_maps)
    outs = []
    for r in res.results:
        o = np.asarray(r["out"], np.float32).reshape(2, 64, 64, 2, 256)
        # (s, c, t, p, x) -> (c, t, p, s, x)
        o = o.transpose(1, 2, 3, 0, 4).reshape(64, 256, 256)
        outs.append(o)
    return np.stack(outs)
